# revision 1
# baseline (speedup 1.0000x reference)
"""Low-rank linear attention (causal, elu+1 feature map) on 8 trn2 cores.

Sharding: core = 2*b + h  (batch b in 0..3, sequence half h in 0..1).
Each core computes out[b, h*2048:(h+1)*2048, :].  Second-half cores
recompute the running K^T.V state over their 2048-token prefix on device
(sel input scales the prefix contribution to zero on first-half cores so
one SPMD program serves all 8 cores).

Phase A: all projections (prefix [V|K] + state accumulation, main
[Q^T;K^T] and [V|K]) — dependency-free dense PE stream.
Phase B: per-chunk causal attention (scores, num/den, state chain,
output projection with the 1/den fold into the PSUM eviction).

Shapes (hardcoded): B=4, S=4096, D=1024, K=64.  L = S/2 = 2048 tokens
per core, processed in 16 chunks of C=128.
"""

import numpy as np

B, S, D, K = 4, 4096, 1024, 64
L = S // 2          # tokens per core (main), also prefix length
C = 128             # chunk (tokens)
G = 512             # token group for P1 matmuls
NCHUNK = L // C     # 16
NGRP = L // G       # 4
NDC = D // 128      # 8 contraction chunks

_cache = {}


def _build_nc():
    import concourse.bacc as bacc
    import concourse.tile as tile
    from concourse import mybir

    f32 = mybir.dt.float32
    bf16 = mybir.dt.bfloat16
    AF = mybir.ActivationFunctionType
    Alu = mybir.AluOpType

    nc = bacc.Bacc()

    xtm = nc.declare_dram_parameter("xtm", [D, L], bf16, isOutput=False)
    xtp = nc.declare_dram_parameter("xtp", [D, L], bf16, isOutput=False)
    wcat = nc.declare_dram_parameter("wcat", [128, 2 * D + C], bf16, isOutput=False)
    wot = nc.declare_dram_parameter("wot", [K, D], bf16, isOutput=False)
    sel = nc.declare_dram_parameter("sel", [C, 1], f32, isOutput=False)
    out = nc.declare_dram_parameter("out", [L, D], f32, isOutput=True)

    with tile.TileContext(nc) as tc:
        with (
            tc.tile_pool(name="consts", bufs=1) as consts,
            tc.tile_pool(name="xm", bufs=1) as xm_pool,
            tc.tile_pool(name="xp", bufs=1) as xp_pool,
            tc.tile_pool(name="small", bufs=4) as small,
            tc.tile_pool(name="vko", bufs=2 * NCHUNK + 1) as vko_pool,
            tc.tile_pool(name="qk", bufs=NCHUNK + 1) as qk_pool,
            tc.tile_pool(name="tmp", bufs=4) as tmp_pool,
            tc.tile_pool(name="ostage", bufs=3) as ostage_pool,
            tc.tile_pool(name="state_pool", bufs=1, space="PSUM") as state_pool,
        ):
            # ---- constants ----
            wcat_sb = consts.tile([128, 2 * D + C], bf16, tag="wcat")
            nc.sync.dma_start(out=wcat_sb, in_=wcat[:, :])
            wqk_sb = [wcat_sb[:, d * 128:(d + 1) * 128] for d in range(NDC)]
            wvk_sb = [wcat_sb[:, D + d * 128:D + (d + 1) * 128] for d in range(NDC)]
            mask_sb = wcat_sb[:, 2 * D:2 * D + C]
            wot_sb = consts.tile([K, D], bf16, tag="wot")
            nc.sync.dma_start(out=wot_sb, in_=wot[:, :])
            sel_sb = consts.tile([C, 1], f32, tag="sel")
            nc.sync.dma_start(out=sel_sb, in_=sel[:, :])
            ones1_sb = consts.tile([1, 1], bf16, tag="ones1")
            nc.vector.memset(ones1_sb, 1.0)
            onesr = consts.tile([1, G], bf16, tag="onesr")
            nc.vector.memset(onesr, 1.0)
            vkbias = consts.tile([1, 2 * K], bf16, tag="vkbias")
            nc.vector.memset(vkbias[:, 0:K], 0.0)
            nc.vector.memset(vkbias[:, K:2 * K], 1.0)
            biasm1 = consts.tile([128, 1], f32, tag="biasm1")
            nc.vector.memset(biasm1, -1.0)

            # ---- x tiles (resident); DMA'd in group sections so early
            # chunks' operands land first and PE ramps immediately ----
            xp_all = []
            xm_all = []
            for d in range(NDC):
                xp_t = xp_pool.tile([128, L], bf16, tag=f"xp{d}")
                xp_all.append(xp_t)
                xm_t = xm_pool.tile([128, L], bf16, tag=f"xm{d}")
                xm_all.append(xm_t)
            for g in range(NGRP):
                gs = slice(g * G, (g + 1) * G)
                for d in range(NDC):
                    nc.sync.dma_start(out=xp_all[d][:, gs],
                                      in_=xtp[d * 128:(d + 1) * 128, gs])
                    nc.sync.dma_start(out=xm_all[d][:, gs],
                                      in_=xtm[d * 128:(d + 1) * 128, gs])

            # running state [K, K+1]: cols 0:K = S[k,m], col K = k_sum.
            state_ps = state_pool.tile([K, 1 + K], f32)

            # =============== PHASE A: projections ===============
            qTs, kTs, vkos = [], [], []
            with (
                tc.tile_pool(name="p1_ps", bufs=3, space="PSUM") as p1_pool,
                tc.tile_pool(name="p2_ps", bufs=4, space="PSUM") as p2_pool,
            ):
                def project_vk(xg, sl):
                    """token-major [V | ones | elu(K)+1] sbuf tile for a chunk."""
                    pp = p2_pool.tile([C, 2 * K], f32, tag="p2")
                    for d in range(NDC):
                        nc.tensor.matmul(pp, xg[d][:, sl], wvk_sb[d],
                                         start=(d == 0), stop=False)
                    nc.tensor.matmul(pp, onesr[:, 0:C], vkbias,
                                     start=False, stop=True)
                    vko = vko_pool.tile([C, 2 * K + 1], bf16, tag="vko")
                    nc.vector.memset(vko[:, K:K + 1], 1.0)
                    nc.vector.tensor_copy(vko[:, 0:K], pp[:, 0:K])
                    eu = tmp_pool.tile([C, K], f32, tag="eu")
                    nc.scalar.activation(eu, pp[:, K:2 * K], AF.Exp, bias=biasm1)
                    nc.vector.scalar_tensor_tensor(
                        vko[:, K + 1:2 * K + 1], eu, 1.0, pp[:, K:2 * K],
                        op0=Alu.min, op1=Alu.max)
                    return vko

                # interleave prefix + main by group so compute follows
                # the DMA section order
                for g in range(NGRP):
                    for c4 in range(G // C):
                        ci = g * (G // C) + c4
                        sl = slice(ci * C, (ci + 1) * C)
                        vko = project_vk(xp_all, sl)
                        vks = vko_pool.tile([C, K + 1], bf16, tag="vks")
                        nc.vector.tensor_scalar_mul(vks, vko[:, 0:K + 1], sel_sb)
                        nc.tensor.matmul(state_ps, vko[:, K + 1:2 * K + 1], vks,
                                         start=(ci == 0), stop=False,
                                         skip_group_check=True)
                    xg = [t[:, g * G:(g + 1) * G] for t in xm_all]
                    p1g = p1_pool.tile([2 * K, G], f32, tag="p1")
                    for d in range(NDC):
                        nc.tensor.matmul(p1g, wqk_sb[d], xg[d],
                                         start=(d == 0), stop=False)
                    nc.tensor.matmul(
                        p1g, ones1_sb[:, 0:1].to_broadcast((1, 2 * K)),
                        onesr, start=False, stop=True)
                    for c4 in range(G // C):
                        sl = slice(c4 * C, (c4 + 1) * C)
                        e1 = tmp_pool.tile([2 * K, C], f32, tag="e1")
                        nc.scalar.activation(e1, p1g[:, sl], AF.Exp, bias=biasm1)
                        qT = qk_pool.tile([K, C], bf16, tag="qT")
                        kT = qk_pool.tile([K, C], bf16, tag="kT")
                        nc.vector.scalar_tensor_tensor(
                            qT, e1[0:K, :], 1.0, p1g[0:K, sl],
                            op0=Alu.min, op1=Alu.max)
                        nc.vector.scalar_tensor_tensor(
                            kT, e1[K:2 * K, :], 1.0, p1g[K:2 * K, sl],
                            op0=Alu.min, op1=Alu.max)
                        qTs.append(qT)
                        kTs.append(kT)
                        vkos.append(project_vk(
                            xm_all,
                            slice(g * G + c4 * C, g * G + (c4 + 1) * C)))

            # sbuf copy of the running state used as matmul lhsT
            ks_sb = small.tile([K, 1 + K], bf16, tag="ks")
            nc.scalar.copy(ks_sb, state_ps)

            # =============== PHASE B: attention ===============
            with tc.tile_pool(name="atnd_ps", bufs=7, space="PSUM") as atnd_pool:
                for ci in range(NCHUNK):
                    qT, kT, vko = qTs[ci], kTs[ci], vkos[ci]
                    # intra-chunk scores A^T[t, s], causal mask
                    at = atnd_pool.tile([C, C], f32, tag="atnd")
                    nc.tensor.matmul(at, kT, qT, start=True, stop=True)
                    atm = tmp_pool.tile([C, C], bf16, tag="atm")
                    nc.vector.tensor_tensor(atm, at, mask_sb, Alu.mult)
                    # [num^T ; den] via lhsT-packed pair
                    nd = atnd_pool.tile([1 + K, C], f32, tag="atnd")
                    nc.tensor.matmul(nd, vko[:, 0:K + 1], atm,
                                     start=True, stop=False)
                    nc.tensor.matmul(nd, ks_sb, qT, start=False, stop=True)
                    # state update + refresh ks_sb
                    nc.tensor.matmul(state_ps, vko[:, K + 1:2 * K + 1],
                                     vko[:, 0:K + 1],
                                     start=False, stop=(ci == NCHUNK - 1),
                                     skip_group_check=True)
                    nc.scalar.copy(ks_sb, state_ps)
                    # reciprocal of den (transpose via 1-row matmul)
                    den_b = small.tile([1, C], bf16, tag="den")
                    nc.vector.tensor_scalar_add(den_b, nd[K:K + 1, :], 1e-6)
                    dtp = atnd_pool.tile([C, 1], f32, tag="atnd")
                    nc.tensor.matmul(dtp, den_b, ones1_sb, start=True, stop=True)
                    recip = small.tile([C, 1], f32, tag="recip")
                    nc.vector.reciprocal(recip, dtp)
                    # output projection; divide by den on PSUM eviction
                    attn = qk_pool.tile([K, C], bf16, tag="attn")
                    nc.vector.tensor_copy(attn, nd[0:K, :])
                    ost = ostage_pool.tile([C, D], f32, tag="ost")
                    for h2 in range(2):
                        op = atnd_pool.tile([C, D // 2], f32, tag="atnd")
                        nc.tensor.matmul(
                            op, attn, wot_sb[:, h2 * 512:(h2 + 1) * 512],
                            start=True, stop=True)
                        nc.scalar.activation(
                            ost[:, h2 * 512:(h2 + 1) * 512], op,
                            AF.Copy, scale=recip)
                    nc.sync.dma_start(out=out[ci * C:(ci + 1) * C, :], in_=ost)

    nc.compile()
    worst = []
    for fn in nc.m.functions:
        for blk in fn.blocks:
            for inst in blk.instructions:
                n = len(inst.sync_info.on_wait) if inst.sync_info else 0
                if n > 1 and type(inst).__name__ == "InstMatmult":
                    worst.append((inst.name, n))
    if worst:
        raise RuntimeError(f"matmuls with >1 wait after lowering: {worst}")
    return nc


def _prep_inputs(x, Wq, Wk, Wv, Wo):
    import ml_dtypes

    bf16 = ml_dtypes.bfloat16
    wqk = np.concatenate([Wq.T, Wk.T], axis=1)                # [D, 2K]
    wvk = np.concatenate([Wv.T, Wk.T], axis=1)                # [D, 2K]
    mask = np.triu(np.ones((C, C), np.float32))               # keep t <= s
    wcat = np.concatenate(
        [wqk[d * 128:(d + 1) * 128, :] for d in range(NDC)]
        + [wvk[d * 128:(d + 1) * 128, :] for d in range(NDC)]
        + [mask],
        axis=1,
    ).astype(bf16)
    wot = np.ascontiguousarray(Wo.T).astype(bf16)             # [K, D]
    zeros_x = np.zeros((D, L), dtype=bf16)
    in_maps = []
    for core in range(8):
        b, h = core // 2, core % 2
        xb = np.ascontiguousarray(x[b].astype(bf16).T)        # [D, S]
        m = {
            "xtm": np.ascontiguousarray(xb[:, h * L:(h + 1) * L]),
            "xtp": np.ascontiguousarray(xb[:, 0:L]) if h else zeros_x,
            "wcat": wcat,
            "wot": wot,
            "sel": np.full((C, 1), float(h), np.float32),
        }
        in_maps.append(m)
    return in_maps


def _run(inputs, trace=False):
    from concourse.bass_utils import run_bass_kernel_spmd

    if "nc" not in _cache:
        _cache["nc"] = _build_nc()
    nc = _cache["nc"]
    in_maps = _prep_inputs(
        np.asarray(inputs["x"], np.float32),
        np.asarray(inputs["Wq"], np.float32),
        np.asarray(inputs["Wk"], np.float32),
        np.asarray(inputs["Wv"], np.float32),
        np.asarray(inputs["Wo"], np.float32),
    )
    res = run_bass_kernel_spmd(nc, in_maps, list(range(8)), trace=trace)
    out = np.empty((B, S, D), np.float32)
    for core in range(8):
        b, h = core // 2, core % 2
        out[b, h * L:(h + 1) * L, :] = res.results[core]["out"]
    return out, res


def kernel(**inputs) -> np.ndarray:
    out, _ = _run(inputs, trace=False)
    return out



# revision 8
# speedup vs baseline: 1.0555x; 1.0555x over previous
"""Low-rank linear attention (causal, elu+1 feature map) on 8 trn2 cores.

Sharding: core = 2*b + h  (batch b in 0..3, sequence half h in 0..1).
Each core computes out[b, h*2048:(h+1)*2048, :].  Second-half cores
recompute the running K^T.V state over their 2048-token prefix on device
(sel scales the prefix contribution to zero on first-half cores so one
SPMD program serves all 8 cores).

v2 layout vs baseline:
  - prefix projections run in fp8(e4m3) DoubleRow (2 k-tiles of 128 per
    matmul, 0.5 cyc/row) with x scaled by 16 and W by 256; prefix x DMA
    is 1 byte/elem.
  - main [Q|K] feature-major + [V|1|K] token-major, both bf16; no bias
    matmuls (the elu+1 "+1" moves to vector-engine evictions).
  - output staged and DMA'd as bf16 (host upcasts).
  - den computed as a PSUM column (two N=1 matmuls) and folded into the
    output-projection eviction as a per-partition scale.
  - Phase B software-pipelined one chunk ahead so the PE queue never
    waits on a vector/scalar round trip.

Shapes (hardcoded): B=4, S=4096, D=1024, K=64.  L = S/2 = 2048 tokens
per core, processed in 16 chunks of C=128.
"""

import numpy as np

B, S, D, K = 4, 4096, 1024, 64
L = S // 2          # tokens per core (main), also prefix length
C = 128             # chunk (tokens)
G = 512             # token group for feature-major matmuls
NCHUNK = L // C     # 16
NGRP = L // G       # 4
NDC = D // 128      # 8 contraction chunks
XS = 16.0           # prefix x fp8 scale
WS = 256.0          # prefix W fp8 scale
PSC = 1.0 / (XS * WS)

_cache = {}


def _build_nc():
    import concourse.bacc as bacc
    import concourse.tile as tile
    from concourse import mybir

    f32 = mybir.dt.float32
    bf16 = mybir.dt.bfloat16
    f8 = mybir.dt.float8e4
    AF = mybir.ActivationFunctionType
    Alu = mybir.AluOpType
    DR = mybir.MatmulPerfMode.DoubleRow

    nc = bacc.Bacc()

    # wcat cols: [wqk 8*128 | wvk 8*128 | mask 128 | ident 128]
    xm = nc.declare_dram_parameter("xm", [D, L], bf16, isOutput=False)
    xp8 = nc.declare_dram_parameter("xp8", [128, 8 * L], f8, isOutput=False)
    wcat = nc.declare_dram_parameter("wcat", [128, 2 * D + 2 * C], bf16,
                                     isOutput=False)
    wkvp = nc.declare_dram_parameter("wkvp", [128, D], f8, isOutput=False)
    wot = nc.declare_dram_parameter("wot", [K, D], bf16, isOutput=False)
    sel = nc.declare_dram_parameter("sel", [C, 1], f32, isOutput=False)
    out = nc.declare_dram_parameter("out", [L, D], bf16, isOutput=True)

    with tile.TileContext(nc) as tc:
        with (
            tc.tile_pool(name="consts", bufs=1) as consts,
            tc.tile_pool(name="xmp", bufs=1) as xm_pool,
            tc.tile_pool(name="xpp", bufs=1) as xp_pool,
            tc.tile_pool(name="proj", bufs=1) as proj_pool,
            tc.tile_pool(name="vko", bufs=NCHUNK) as vko_pool,
            tc.tile_pool(name="kvp", bufs=2) as kvp_pool,
            tc.tile_pool(name="kvpt", bufs=4) as kvpt_pool,
            tc.tile_pool(name="small", bufs=6) as small,
            tc.tile_pool(name="tmp", bufs=6) as tmp_pool,
            tc.tile_pool(name="ostage", bufs=3) as ostage_pool,
            tc.tile_pool(name="state_pool", bufs=1, space="PSUM") as state_pool,
        ):
            # ---- constants ----
            wcat_sb = consts.tile([128, 2 * D + 2 * C], bf16, tag="wcat")
            for s in range(18):
                nc.sync.dma_start(out=wcat_sb[:, s * 128:(s + 1) * 128],
                                  in_=wcat[:, s * 128:(s + 1) * 128])
            wqk_sb = [wcat_sb[:, d * 128:(d + 1) * 128] for d in range(NDC)]
            wvk_sb = [wcat_sb[:, D + d * 128:D + (d + 1) * 128]
                      for d in range(NDC)]
            mask_sb = wcat_sb[:, 2 * D:2 * D + C]
            ident_sb = wcat_sb[:, 2 * D + C:2 * D + 2 * C]
            wkvp_sb = consts.tile([128, 4, 2, 128], f8, tag="wkvp")
            nc.sync.dma_start(out=wkvp_sb, in_=wkvp[:, :])
            wot_sb = consts.tile([K, D], bf16, tag="wot")
            nc.sync.dma_start(out=wot_sb, in_=wot[:, :])
            sel_sb = consts.tile([C, 1], f32, tag="sel")
            nc.sync.dma_start(out=sel_sb, in_=sel[:, :])
            onec_sb = consts.tile([C, 1], bf16, tag="onec")
            nc.vector.memset(onec_sb, 1.0)

            # ---- x tiles (resident); DMA'd per group so early groups'
            # operands land first and the PE ramps immediately ----
            xm_all = [xm_pool.tile([128, L], bf16, tag=f"xm{d}", name=f"xm{d}")
                      for d in range(NDC)]
            xp_sb = xp_pool.tile([128, 8, L], f8, tag="xp")
            for g in range(NGRP):
                gs = slice(g * G, (g + 1) * G)
                for d in range(NDC):
                    nc.sync.dma_start(out=xm_all[d][:, gs],
                                      in_=xm[d * 128:(d + 1) * 128, gs])
                for d in range(NDC):
                    nc.sync.dma_start(
                        out=xp_sb[:, d, gs],
                        in_=xp8[:, d * L + g * G:d * L + (g + 1) * G])

            # persistent sbuf
            qT_sb = proj_pool.tile([K, L], bf16, tag="qT")
            kT_sb = proj_pool.tile([K, L], bf16, tag="kT")
            vkos = [vko_pool.tile([C, 130], bf16, tag=f"vko{i}", name=f"vko{i}")
                    for i in range(NCHUNK)]
            ks_sb = small.tile([K, K + 1], bf16, tag="ks")

            # running state [K, K+1]: cols 0:K = S[k,m], col K = k_sum.
            state_ps = state_pool.tile([K, 1 + K], f32)

            # =============== PHASE A: projections ===============
            kvp_tiles = [None] * NGRP

            with (
                tc.tile_pool(name="p1_ps", bufs=2, space="PSUM") as p1_pool,
                tc.tile_pool(name="pkv_ps", bufs=1, space="PSUM") as pkv_pool,
                tc.tile_pool(name="pp_ps", bufs=2, space="PSUM") as pp_pool,
                tc.tile_pool(name="ptr_ps", bufs=2, space="PSUM") as ptr_pool,
            ):
                def prefix_tail(g):
                    """transpose + state accumulation for prefix group g."""
                    kvp_g = kvp_tiles[g]
                    for c4 in range(G // C):
                        ci = g * (G // C) + c4
                        ptr = ptr_pool.tile([C, 128], bf16, tag="ptr", name="ptr")
                        nc.tensor.transpose(
                            ptr, kvp_g[:, c4 * C:(c4 + 1) * C], ident_sb)
                        kvpt = kvpt_pool.tile([C, 129], bf16, tag="kvpt", name="kvpt")
                        nc.vector.tensor_copy(kvpt[:, 0:128], ptr)
                        nc.vector.memset(kvpt[:, 128:129], 1.0)
                        vks = kvpt_pool.tile([C, K + 1], bf16, tag="vks", name="vks")
                        nc.vector.tensor_scalar_mul(
                            vks, kvpt[:, K:2 * K + 1], sel_sb)
                        nc.tensor.matmul(state_ps, kvpt[:, 0:K], vks,
                                         start=(ci == 0), stop=False,
                                         skip_group_check=True)

                for g in range(NGRP):
                    gs = slice(g * G, (g + 1) * G)
                    # ---- [Q|K] feature-major, bf16 ----
                    p1 = p1_pool.tile([128, G], f32, tag="p1")
                    for d in range(NDC):
                        nc.tensor.matmul(p1, wqk_sb[d], xm_all[d][:, gs],
                                         start=(d == 0), stop=(d == NDC - 1))
                    e1 = tmp_pool.tile([128, G], f32, tag="e1")
                    nc.scalar.activation(e1, p1, AF.Exp)
                    p1t = tmp_pool.tile([128, G], f32, tag="p1t")
                    nc.vector.tensor_scalar_add(p1t, p1, 1.0)
                    nc.vector.scalar_tensor_tensor(
                        qT_sb[:, gs], e1[0:K, :], 1.0, p1t[0:K, :],
                        op0=Alu.min, op1=Alu.max)
                    nc.vector.scalar_tensor_tensor(
                        kT_sb[:, gs], e1[K:2 * K, :], 1.0, p1t[K:2 * K, :],
                        op0=Alu.min, op1=Alu.max)

                    # ---- [V|1|K] token-major, bf16 ----
                    for c4 in range(G // C):
                        ci = g * (G // C) + c4
                        sl = slice(ci * C, (ci + 1) * C)
                        pp = pp_pool.tile([C, 128], f32, tag="pp")
                        for d in range(NDC):
                            nc.tensor.matmul(pp, xm_all[d][:, sl], wvk_sb[d],
                                             start=(d == 0),
                                             stop=(d == NDC - 1))
                        vko = vkos[ci]
                        nc.vector.tensor_copy(vko[:, 0:K], pp[:, 0:K])
                        nc.vector.memset(vko[:, K:K + 1], 1.0)
                        e3 = tmp_pool.tile([C, K], f32, tag="e3")
                        nc.scalar.activation(e3, pp[:, K:2 * K], AF.Exp)
                        p3 = tmp_pool.tile([C, K], f32, tag="p3")
                        nc.vector.tensor_scalar_add(p3, pp[:, K:2 * K], 1.0)
                        nc.vector.scalar_tensor_tensor(
                            vko[:, K + 1:2 * K + 1], e3, 1.0, p3,
                            op0=Alu.min, op1=Alu.max)

                    # ---- prefix [K|V] feature-major, fp8 DoubleRow ----
                    pkv = pkv_pool.tile([128, G], f32, tag="pkv")
                    for c in range(4):
                        nc.tensor.matmul(pkv, wkvp_sb[:, c],
                                         xp_sb[:, 2 * c:2 * c + 2, gs],
                                         start=(c == 0), stop=(c == 3),
                                         perf_mode=DR)
                    kvp_g = kvp_pool.tile([128, G], bf16, tag="kvp")
                    kvp_tiles[g] = kvp_g
                    e2 = tmp_pool.tile([K, G], f32, tag="e2")
                    nc.scalar.activation(e2, pkv[0:K, :], AF.Exp, scale=PSC)
                    p2 = tmp_pool.tile([K, G], f32, tag="p2")
                    nc.vector.tensor_scalar(p2, pkv[0:K, :], PSC, 1.0,
                                            op0=Alu.mult, op1=Alu.add)
                    nc.vector.scalar_tensor_tensor(
                        kvp_g[0:K, :], e2, 1.0, p2, op0=Alu.min, op1=Alu.max)
                    nc.vector.tensor_scalar_mul(
                        kvp_g[K:128, :], pkv[K:128, :], PSC)

                    # ---- lagged prefix transpose/state for group g-1 ----
                    if g > 0:
                        prefix_tail(g - 1)
                prefix_tail(NGRP - 1)

            # sbuf copy of the running state used as matmul lhsT
            nc.scalar.copy(ks_sb, state_ps)

            # =============== PHASE B: attention ===============
            with (
                tc.tile_pool(name="andc_ps", bufs=4, space="PSUM") as andc_pool,
                tc.tile_pool(name="op_ps", bufs=3, space="PSUM") as op_pool,
            ):
                at_pool = nd_pool = dc_pool = andc_pool
                ats = [None] * NCHUNK
                atms = [None] * NCHUNK
                nds = [None] * NCHUNK
                dcs = [None] * NCHUNK
                recs = [None] * NCHUNK
                attns = [None] * NCHUNK
                osts = [None] * NCHUNK

                def sc(i):
                    sl = slice(i * C, (i + 1) * C)
                    ats[i] = at_pool.tile([C, C], f32, tag="andc", name="at")
                    nc.tensor.matmul(ats[i], kT_sb[:, sl], qT_sb[:, sl],
                                     start=True, stop=True)

                def vecatm(i):
                    atms[i] = tmp_pool.tile([C, C], bf16, tag="atm", name="atm")
                    nc.vector.tensor_tensor(atms[i], ats[i], mask_sb, Alu.mult)

                def nd(i):
                    sl = slice(i * C, (i + 1) * C)
                    nds[i] = nd_pool.tile([K, C], f32, tag="andc", name="nd")
                    nc.tensor.matmul(nds[i], vkos[i][:, 0:K], atms[i],
                                     start=True, stop=False)
                    nc.tensor.matmul(nds[i], ks_sb[:, 0:K], qT_sb[:, sl],
                                     start=False, stop=True)
                    dcs[i] = dc_pool.tile([C, 1], f32, tag="andc", name="dc")
                    nc.tensor.matmul(dcs[i], atms[i], onec_sb,
                                     start=True, stop=False)
                    nc.tensor.matmul(dcs[i], qT_sb[:, sl], ks_sb[:, K:K + 1],
                                     start=False, stop=True)

                def st(i):
                    nc.tensor.matmul(state_ps, vkos[i][:, K + 1:2 * K + 1],
                                     vkos[i][:, 0:K + 1],
                                     start=False, stop=(i == NCHUNK - 1),
                                     skip_group_check=True)

                def ksc(i):
                    if i < NCHUNK - 1:
                        nc.scalar.copy(ks_sb, state_ps)

                def recattn(i):
                    den = small.tile([C, 1], f32, tag="den", name="den")
                    nc.vector.tensor_scalar_add(den, dcs[i], 1e-6)
                    recs[i] = small.tile([C, 1], f32, tag="rec", name="rec")
                    nc.vector.reciprocal(recs[i], den)
                    attns[i] = small.tile([K, C], bf16, tag="attn", name="attn")
                    nc.vector.tensor_copy(attns[i], nds[i])

                def op(i):
                    osts[i] = ostage_pool.tile([C, D], bf16, tag="ost", name="ost")
                    for h2 in range(2):
                        o = op_pool.tile([C, D // 2], f32, tag="op", name="op")
                        nc.tensor.matmul(
                            o, attns[i], wot_sb[:, h2 * 512:(h2 + 1) * 512],
                            start=True, stop=True)
                        if h2 == 0:
                            nc.scalar.activation(
                                osts[i][:, h2 * 512:(h2 + 1) * 512], o,
                                AF.Copy, scale=recs[i])
                        else:
                            nc.vector.tensor_scalar_mul(
                                osts[i][:, h2 * 512:(h2 + 1) * 512], o,
                                recs[i])
                    nc.sync.dma_start(out=out[i * C:(i + 1) * C, :],
                                      in_=osts[i])

                sc(0)
                vecatm(0)
                for i in range(NCHUNK):
                    nd(i)
                    st(i)
                    if i + 1 < NCHUNK:
                        sc(i + 1)
                    recattn(i)
                    if i + 1 < NCHUNK:
                        vecatm(i + 1)
                    ksc(i)
                    if i > 0:
                        op(i - 1)
                op(NCHUNK - 1)

    nc.compile()
    worst = []
    for fn in nc.m.functions:
        for blk in fn.blocks:
            for inst in blk.instructions:
                n = len(inst.sync_info.on_wait) if inst.sync_info else 0
                if n > 1 and type(inst).__name__ == "InstMatmult":
                    worst.append((inst.name, n))
    if worst:
        import sys
        print(f"WARN: matmuls with >1 wait after lowering: {worst}",
              file=sys.stderr)
    return nc


def _prep_inputs(x, Wq, Wk, Wv, Wo):
    import ml_dtypes

    bf16 = ml_dtypes.bfloat16
    f8 = ml_dtypes.float8_e4m3

    def dmajor(Wcat):
        # [p, d*128+m] = Wcat[m, 128*d+p]
        return np.ascontiguousarray(
            Wcat.T.reshape(NDC, 128, 128).transpose(1, 0, 2).reshape(128, D))

    wqk = dmajor(np.concatenate([Wq, Wk], axis=0))           # [Q|K]
    wvk = dmajor(np.concatenate([Wv, Wk], axis=0))           # [V|K]
    mask = np.triu(np.ones((C, C), np.float32))              # keep s <= t
    ident = np.eye(C, dtype=np.float32)
    wcat = np.concatenate([wqk, wvk, mask, ident], axis=1).astype(bf16)
    # prefix weights: [K|V], fp8, x256, DoubleRow layout [p, c, i, m]
    wkv = (WS * np.concatenate([Wk, Wv], axis=0)).T          # [D, 128]
    wkvp = np.ascontiguousarray(
        wkv.reshape(4, 2, 128, 128).transpose(2, 0, 1, 3).reshape(128, D)
    ).astype(f8)
    wot = np.ascontiguousarray(Wo.T).astype(bf16)            # [K, D]
    zeros_xp = np.zeros((128, 8 * L), dtype=f8)
    in_maps = []
    for core in range(8):
        b, h = core // 2, core % 2
        xb = np.ascontiguousarray(x[b].astype(bf16).T)       # [D, S]
        if h:
            xp = np.ascontiguousarray(
                (XS * x[b, 0:L].T.astype(np.float32))
                .reshape(NDC, 128, L).transpose(1, 0, 2).reshape(128, 8 * L)
            ).astype(f8)
        else:
            xp = zeros_xp
        m = {
            "xm": np.ascontiguousarray(xb[:, h * L:(h + 1) * L]),
            "xp8": xp,
            "wcat": wcat,
            "wkvp": wkvp,
            "wot": wot,
            "sel": np.full((C, 1), float(h), np.float32),
        }
        in_maps.append(m)
    return in_maps


def _run(inputs, trace=False):
    from concourse.bass_utils import run_bass_kernel_spmd

    if "nc" not in _cache:
        _cache["nc"] = _build_nc()
    nc = _cache["nc"]
    in_maps = _prep_inputs(
        np.asarray(inputs["x"], np.float32),
        np.asarray(inputs["Wq"], np.float32),
        np.asarray(inputs["Wk"], np.float32),
        np.asarray(inputs["Wv"], np.float32),
        np.asarray(inputs["Wo"], np.float32),
    )
    res = run_bass_kernel_spmd(nc, in_maps, list(range(8)), trace=trace)
    out = np.empty((B, S, D), np.float32)
    for core in range(8):
        b, h = core // 2, core % 2
        out[b, h * L:(h + 1) * L, :] = res.results[core]["out"].astype(
            np.float32)
    return out, res


def kernel(**inputs) -> np.ndarray:
    out, _ = _run(inputs, trace=False)
    return out


# revision 9
# speedup vs baseline: 1.2008x; 1.1377x over previous
"""Low-rank linear attention (causal, elu+1 feature map) on 8 trn2 cores.

Sharding: core = 2*b + h  (batch b in 0..3, sequence half h in 0..1).
Each core computes out[b, h*2048:(h+1)*2048, :].  Second-half cores
recompute the running K^T.V state over their 2048-token prefix on device
(sel scales the prefix contribution to zero on first-half cores so one
SPMD program serves all 8 cores).

v2 layout vs baseline:
  - prefix projections run in fp8(e4m3) DoubleRow (2 k-tiles of 128 per
    matmul, 0.5 cyc/row) with x scaled by 16 and W by 256; prefix x DMA
    is 1 byte/elem.
  - main [Q|K] feature-major + [V|1|K] token-major, both bf16; no bias
    matmuls (the elu+1 "+1" moves to vector-engine evictions).
  - output staged and DMA'd as bf16 (host upcasts).
  - den computed as a PSUM column (two N=1 matmuls) and folded into the
    output-projection eviction as a per-partition scale.
  - Phase B software-pipelined one chunk ahead so the PE queue never
    waits on a vector/scalar round trip.

Shapes (hardcoded): B=4, S=4096, D=1024, K=64.  L = S/2 = 2048 tokens
per core, processed in 16 chunks of C=128.
"""

import numpy as np

B, S, D, K = 4, 4096, 1024, 64
L = S // 2          # tokens per core (main), also prefix length
C = 128             # chunk (tokens)
G = 512             # token group for feature-major matmuls
NCHUNK = L // C     # 16
NGRP = L // G       # 4
NDC = D // 128      # 8 contraction chunks
XS = 16.0           # prefix x fp8 scale
WS = 256.0          # prefix W fp8 scale
PSC = 1.0 / (XS * WS)

_cache = {}


def _build_nc():
    import concourse.bacc as bacc
    import concourse.tile as tile
    from concourse import mybir

    f32 = mybir.dt.float32
    bf16 = mybir.dt.bfloat16
    f8 = mybir.dt.float8e4
    AF = mybir.ActivationFunctionType
    Alu = mybir.AluOpType
    DR = mybir.MatmulPerfMode.DoubleRow

    nc = bacc.Bacc()

    # x params are host-laid-out [p][d][c] so each DMA row is contiguous
    xm = nc.declare_dram_parameter("xm", [128, 8 * L], bf16, isOutput=False)
    xp8 = nc.declare_dram_parameter("xp8", [128, 8 * L], f8, isOutput=False)
    wqkm = nc.declare_dram_parameter("wqkm", [128, D + C], bf16,
                                     isOutput=False)
    wvki = nc.declare_dram_parameter("wvki", [128, D + C], bf16,
                                     isOutput=False)
    wkvp = nc.declare_dram_parameter("wkvp", [128, D], f8, isOutput=False)
    wot = nc.declare_dram_parameter("wot", [K, D], bf16, isOutput=False)
    sel = nc.declare_dram_parameter("sel", [C, 1], f32, isOutput=False)
    out = nc.declare_dram_parameter("out", [L, D], bf16, isOutput=True)

    with tile.TileContext(nc) as tc:
        with (
            tc.tile_pool(name="consts", bufs=1) as consts,
            tc.tile_pool(name="xmp", bufs=1) as xm_pool,
            tc.tile_pool(name="xpp", bufs=1) as xp_pool,
            tc.tile_pool(name="proj", bufs=1) as proj_pool,
            tc.tile_pool(name="vko", bufs=NCHUNK) as vko_pool,
            tc.tile_pool(name="kvp", bufs=2) as kvp_pool,
            tc.tile_pool(name="kvpt", bufs=4) as kvpt_pool,
            tc.tile_pool(name="small", bufs=6) as small,
            tc.tile_pool(name="tmp", bufs=6) as tmp_pool,
            tc.tile_pool(name="ostage", bufs=3) as ostage_pool,
            tc.tile_pool(name="state_pool", bufs=1, space="PSUM") as state_pool,
        ):
            # ---- constants + x (few fat DMA calls; rows are multi-KB
            # contiguous so the hw spreads full-rate descriptors over all
            # 16 queues; SP issue cost is ~0.5us per call) ----
            wqkm_sb = consts.tile([128, D + C], bf16, tag="wqkm")
            wvki_sb = consts.tile([128, D + C], bf16, tag="wvki")
            wkvp_sb = consts.tile([128, 4, 2, 128], f8, tag="wkvp")
            wot_sb = consts.tile([K, D], bf16, tag="wot")
            sel_sb = consts.tile([C, 1], f32, tag="sel")
            xm3 = xm_pool.tile([128, 8, L], bf16, tag="xm3")
            xp_sb = xp_pool.tile([128, 8, L], f8, tag="xp")
            xmv = xm[:, :].rearrange("p (d c) -> p d c", d=8)
            xpv = xp8[:, :].rearrange("p (d c) -> p d c", d=8)

            nc.sync.dma_start(out=wqkm_sb, in_=wqkm[:, :])
            nc.sync.dma_start(out=xm3[:, :, 0:D], in_=xmv[:, :, 0:D])
            nc.sync.dma_start(out=wvki_sb, in_=wvki[:, :])
            nc.sync.dma_start(out=xp_sb, in_=xpv)
            nc.sync.dma_start(out=wkvp_sb, in_=wkvp[:, :])
            nc.sync.dma_start(out=xm3[:, :, D:L], in_=xmv[:, :, D:L])
            nc.sync.dma_start(out=wot_sb, in_=wot[:, :])
            nc.sync.dma_start(out=sel_sb, in_=sel[:, :])

            wqk_sb = [wqkm_sb[:, d * 128:(d + 1) * 128] for d in range(NDC)]
            wvk_sb = [wvki_sb[:, d * 128:(d + 1) * 128] for d in range(NDC)]
            mask_sb = wqkm_sb[:, D:D + C]
            ident_sb = wvki_sb[:, D:D + C]
            onec_sb = consts.tile([C, 1], bf16, tag="onec")
            nc.vector.memset(onec_sb, 1.0)

            # persistent sbuf
            qT_sb = proj_pool.tile([K, L], bf16, tag="qT")
            kT_sb = proj_pool.tile([K, L], bf16, tag="kT")
            vkos = [vko_pool.tile([C, 130], bf16, tag=f"vko{i}", name=f"vko{i}")
                    for i in range(NCHUNK)]
            ks_sb = small.tile([K, K + 1], bf16, tag="ks")

            # running state [K, K+1]: cols 0:K = S[k,m], col K = k_sum.
            state_ps = state_pool.tile([K, 1 + K], f32)

            # =============== PHASE A: projections ===============
            kvp_tiles = [None] * NGRP

            with (
                tc.tile_pool(name="p1_ps", bufs=2, space="PSUM") as p1_pool,
                tc.tile_pool(name="pkv_ps", bufs=1, space="PSUM") as pkv_pool,
                tc.tile_pool(name="pp_ps", bufs=2, space="PSUM") as pp_pool,
                tc.tile_pool(name="ptr_ps", bufs=2, space="PSUM") as ptr_pool,
            ):
                def prefix_tail(g):
                    """transpose + state accumulation for prefix group g."""
                    kvp_g = kvp_tiles[g]
                    for c4 in range(G // C):
                        ci = g * (G // C) + c4
                        ptr = ptr_pool.tile([C, 128], bf16, tag="ptr", name="ptr")
                        nc.tensor.transpose(
                            ptr, kvp_g[:, c4 * C:(c4 + 1) * C], ident_sb)
                        kvpt = kvpt_pool.tile([C, 129], bf16, tag="kvpt", name="kvpt")
                        nc.vector.tensor_copy(kvpt[:, 0:128], ptr)
                        nc.vector.memset(kvpt[:, 128:129], 1.0)
                        vks = kvpt_pool.tile([C, K + 1], bf16, tag="vks", name="vks")
                        nc.vector.tensor_scalar_mul(
                            vks, kvpt[:, K:2 * K + 1], sel_sb)
                        nc.tensor.matmul(state_ps, kvpt[:, 0:K], vks,
                                         start=(ci == 0), stop=False,
                                         skip_group_check=True)

                for g in range(NGRP):
                    gs = slice(g * G, (g + 1) * G)
                    # ---- [Q|K] feature-major, bf16 ----
                    p1 = p1_pool.tile([128, G], f32, tag="p1")
                    for d in range(NDC):
                        nc.tensor.matmul(p1, wqk_sb[d], xm3[:, d, gs],
                                         start=(d == 0), stop=(d == NDC - 1))
                    e1 = tmp_pool.tile([128, G], f32, tag="e1")
                    nc.scalar.activation(e1, p1, AF.Exp)
                    p1t = tmp_pool.tile([128, G], f32, tag="p1t")
                    nc.vector.tensor_scalar_add(p1t, p1, 1.0)
                    nc.vector.scalar_tensor_tensor(
                        qT_sb[:, gs], e1[0:K, :], 1.0, p1t[0:K, :],
                        op0=Alu.min, op1=Alu.max)
                    nc.vector.scalar_tensor_tensor(
                        kT_sb[:, gs], e1[K:2 * K, :], 1.0, p1t[K:2 * K, :],
                        op0=Alu.min, op1=Alu.max)

                    # ---- [V|1|K] token-major, bf16 ----
                    for c4 in range(G // C):
                        ci = g * (G // C) + c4
                        sl = slice(ci * C, (ci + 1) * C)
                        pp = pp_pool.tile([C, 128], f32, tag="pp")
                        for d in range(NDC):
                            nc.tensor.matmul(pp, xm3[:, d, sl], wvk_sb[d],
                                             start=(d == 0),
                                             stop=(d == NDC - 1))
                        vko = vkos[ci]
                        nc.vector.tensor_copy(vko[:, 0:K], pp[:, 0:K])
                        nc.vector.memset(vko[:, K:K + 1], 1.0)
                        e3 = tmp_pool.tile([C, K], f32, tag="e3")
                        nc.scalar.activation(e3, pp[:, K:2 * K], AF.Exp)
                        p3 = tmp_pool.tile([C, K], f32, tag="p3")
                        nc.vector.tensor_scalar_add(p3, pp[:, K:2 * K], 1.0)
                        nc.vector.scalar_tensor_tensor(
                            vko[:, K + 1:2 * K + 1], e3, 1.0, p3,
                            op0=Alu.min, op1=Alu.max)

                    # ---- prefix [K|V] feature-major, fp8 DoubleRow ----
                    pkv = pkv_pool.tile([128, G], f32, tag="pkv")
                    for c in range(4):
                        nc.tensor.matmul(pkv, wkvp_sb[:, c],
                                         xp_sb[:, 2 * c:2 * c + 2, gs],
                                         start=(c == 0), stop=(c == 3),
                                         perf_mode=DR)
                    kvp_g = kvp_pool.tile([128, G], bf16, tag="kvp")
                    kvp_tiles[g] = kvp_g
                    e2 = tmp_pool.tile([K, G], f32, tag="e2")
                    nc.scalar.activation(e2, pkv[0:K, :], AF.Exp, scale=PSC)
                    p2 = tmp_pool.tile([K, G], f32, tag="p2")
                    nc.vector.tensor_scalar(p2, pkv[0:K, :], PSC, 1.0,
                                            op0=Alu.mult, op1=Alu.add)
                    nc.vector.scalar_tensor_tensor(
                        kvp_g[0:K, :], e2, 1.0, p2, op0=Alu.min, op1=Alu.max)
                    nc.vector.tensor_scalar_mul(
                        kvp_g[K:128, :], pkv[K:128, :], PSC)

                    # ---- lagged prefix transpose/state for group g-1 ----
                    if g > 0:
                        prefix_tail(g - 1)
                prefix_tail(NGRP - 1)

            # sbuf copy of the running state used as matmul lhsT
            nc.scalar.copy(ks_sb, state_ps)

            # =============== PHASE B: attention ===============
            with (
                tc.tile_pool(name="andc_ps", bufs=4, space="PSUM") as andc_pool,
                tc.tile_pool(name="op_ps", bufs=3, space="PSUM") as op_pool,
            ):
                at_pool = nd_pool = dc_pool = andc_pool
                ats = [None] * NCHUNK
                atms = [None] * NCHUNK
                nds = [None] * NCHUNK
                dcs = [None] * NCHUNK
                recs = [None] * NCHUNK
                attns = [None] * NCHUNK
                osts = [None] * NCHUNK

                def sc(i):
                    sl = slice(i * C, (i + 1) * C)
                    ats[i] = at_pool.tile([C, C], f32, tag="andc", name="at")
                    nc.tensor.matmul(ats[i], kT_sb[:, sl], qT_sb[:, sl],
                                     start=True, stop=True)

                def vecatm(i):
                    atms[i] = tmp_pool.tile([C, C], bf16, tag="atm", name="atm")
                    nc.vector.tensor_tensor(atms[i], ats[i], mask_sb, Alu.mult)

                def nd(i):
                    sl = slice(i * C, (i + 1) * C)
                    nds[i] = nd_pool.tile([K, C], f32, tag="andc", name="nd")
                    nc.tensor.matmul(nds[i], vkos[i][:, 0:K], atms[i],
                                     start=True, stop=False)
                    nc.tensor.matmul(nds[i], ks_sb[:, 0:K], qT_sb[:, sl],
                                     start=False, stop=True)
                    dcs[i] = dc_pool.tile([C, 1], f32, tag="andc", name="dc")
                    nc.tensor.matmul(dcs[i], atms[i], onec_sb,
                                     start=True, stop=False)
                    nc.tensor.matmul(dcs[i], qT_sb[:, sl], ks_sb[:, K:K + 1],
                                     start=False, stop=True)

                def st(i):
                    nc.tensor.matmul(state_ps, vkos[i][:, K + 1:2 * K + 1],
                                     vkos[i][:, 0:K + 1],
                                     start=False, stop=(i == NCHUNK - 1),
                                     skip_group_check=True)

                def ksc(i):
                    if i < NCHUNK - 1:
                        nc.scalar.copy(ks_sb, state_ps)

                def recattn(i):
                    den = small.tile([C, 1], f32, tag="den", name="den")
                    nc.vector.tensor_scalar_add(den, dcs[i], 1e-6)
                    recs[i] = small.tile([C, 1], f32, tag="rec", name="rec")
                    nc.vector.reciprocal(recs[i], den)
                    attns[i] = small.tile([K, C], bf16, tag="attn", name="attn")
                    nc.vector.tensor_copy(attns[i], nds[i])

                def op(i):
                    osts[i] = ostage_pool.tile([C, D], bf16, tag="ost", name="ost")
                    for h2 in range(2):
                        o = op_pool.tile([C, D // 2], f32, tag="op", name="op")
                        nc.tensor.matmul(
                            o, attns[i], wot_sb[:, h2 * 512:(h2 + 1) * 512],
                            start=True, stop=True)
                        if h2 == 0:
                            nc.scalar.activation(
                                osts[i][:, h2 * 512:(h2 + 1) * 512], o,
                                AF.Copy, scale=recs[i])
                        else:
                            nc.vector.tensor_scalar_mul(
                                osts[i][:, h2 * 512:(h2 + 1) * 512], o,
                                recs[i])
                    nc.gpsimd.dma_start(out=out[i * C:(i + 1) * C, :],
                                        in_=osts[i])

                sc(0)
                vecatm(0)
                for i in range(NCHUNK):
                    nd(i)
                    st(i)
                    if i + 1 < NCHUNK:
                        sc(i + 1)
                    recattn(i)
                    if i + 1 < NCHUNK:
                        vecatm(i + 1)
                    ksc(i)
                    if i > 0:
                        op(i - 1)
                op(NCHUNK - 1)

    nc.compile()
    worst = []
    for fn in nc.m.functions:
        for blk in fn.blocks:
            for inst in blk.instructions:
                n = len(inst.sync_info.on_wait) if inst.sync_info else 0
                if n > 1 and type(inst).__name__ == "InstMatmult":
                    worst.append((inst.name, n))
    if worst:
        import sys
        print(f"WARN: matmuls with >1 wait after lowering: {worst}",
              file=sys.stderr)
    return nc


def _prep_inputs(x, Wq, Wk, Wv, Wo):
    import ml_dtypes

    bf16 = ml_dtypes.bfloat16
    f8 = ml_dtypes.float8_e4m3

    def dmajor(Wcat):
        # [p, d*128+m] = Wcat[m, 128*d+p]
        return np.ascontiguousarray(
            Wcat.T.reshape(NDC, 128, 128).transpose(1, 0, 2).reshape(128, D))

    wqk = dmajor(np.concatenate([Wq, Wk], axis=0))           # [Q|K]
    wvk = dmajor(np.concatenate([Wv, Wk], axis=0))           # [V|K]
    mask = np.triu(np.ones((C, C), np.float32))              # keep s <= t
    ident = np.eye(C, dtype=np.float32)
    wqkm = np.concatenate([wqk, mask], axis=1).astype(bf16)
    wvki = np.concatenate([wvk, ident], axis=1).astype(bf16)
    # prefix weights: [K|V], fp8, x256, DoubleRow layout [p, c, i, m]
    wkv = (WS * np.concatenate([Wk, Wv], axis=0)).T          # [D, 128]
    wkvp = np.ascontiguousarray(
        wkv.reshape(4, 2, 128, 128).transpose(2, 0, 1, 3).reshape(128, D)
    ).astype(f8)
    wot = np.ascontiguousarray(Wo.T).astype(bf16)            # [K, D]
    zeros_xp = np.zeros((128, 8 * L), dtype=f8)

    def pdc(xt):
        # [D, L] -> [p][d][c] flat [128, 8*L]
        return np.ascontiguousarray(
            xt.reshape(NDC, 128, L).transpose(1, 0, 2).reshape(128, 8 * L))

    in_maps = []
    for core in range(8):
        b, h = core // 2, core % 2
        xb = x[b].T                                          # [D, S]
        if h:
            xp = pdc(XS * xb[:, 0:L]).astype(f8)
        else:
            xp = zeros_xp
        m = {
            "xm": pdc(xb[:, h * L:(h + 1) * L].astype(bf16)),
            "xp8": xp,
            "wqkm": wqkm,
            "wvki": wvki,
            "wkvp": wkvp,
            "wot": wot,
            "sel": np.full((C, 1), float(h), np.float32),
        }
        in_maps.append(m)
    return in_maps


def _run(inputs, trace=False):
    from concourse.bass_utils import run_bass_kernel_spmd

    if "nc" not in _cache:
        _cache["nc"] = _build_nc()
    nc = _cache["nc"]
    in_maps = _prep_inputs(
        np.asarray(inputs["x"], np.float32),
        np.asarray(inputs["Wq"], np.float32),
        np.asarray(inputs["Wk"], np.float32),
        np.asarray(inputs["Wv"], np.float32),
        np.asarray(inputs["Wo"], np.float32),
    )
    res = run_bass_kernel_spmd(nc, in_maps, list(range(8)), trace=trace)
    out = np.empty((B, S, D), np.float32)
    for core in range(8):
        b, h = core // 2, core % 2
        out[b, h * L:(h + 1) * L, :] = res.results[core]["out"].astype(
            np.float32)
    return out, res


def kernel(**inputs) -> np.ndarray:
    out, _ = _run(inputs, trace=False)
    return out


# revision 13
# speedup vs baseline: 1.2908x; 1.0750x over previous
"""Low-rank linear attention (causal, elu+1 feature map) on 8 trn2 cores.

Sharding: core = 2*b + h  (batch b in 0..3, sequence half h in 0..1).
Each core computes out[b, h*2048:(h+1)*2048, :].  Second-half cores
recompute the running K^T.V state over their 2048-token prefix on device
(sel scales the prefix contribution to zero on first-half cores so one
SPMD program serves all 8 cores).

v4 notes:
  - few fat DMA calls (multi-KB contiguous rows spread row-descriptors
    over all 16 queues; each dma_start costs ~0.5us of issue time).
  - all three projection passes stream rhs at full 128-row contraction;
    prefix runs token-major in fp8 e4m3 (x16 / W256 scaling) so no
    transposes are needed.
  - elu+1's "+1" lands in PSUM via ones-outer-product matmuls so
    evictions read PSUM directly.
  - every Phase B matmul is zero-padded to 128 contraction rows: the
    PE HAM clock gate watches array activity, and 64-row matmuls leave
    the clock throttled at 1.2 GHz.
  - 1/den folds into the output-projection evictions (per-partition
    scale on scalar + vector engines).

Shapes (hardcoded): B=4, S=4096, D=1024, K=64.  L = S/2 = 2048 tokens
per core, processed in 16 chunks of C=128.
"""

import numpy as np

B, S, D, K = 4, 4096, 1024, 64
L = S // 2          # tokens per core (main), also prefix length
C = 128             # chunk (tokens)
G = 512             # token group (4 chunks share one PSUM bank / evictions)
NCHUNK = L // C     # 16
NGRP = L // G       # 4
NDC = D // 128      # 8 contraction chunks
XS = 16.0           # prefix x fp8 scale
WS = 256.0          # prefix W fp8 scale
PSC = 1.0 / (XS * WS)

_cache = {}


def _build_nc():
    import concourse.bacc as bacc
    import concourse.tile as tile
    from concourse import mybir

    f32 = mybir.dt.float32
    bf16 = mybir.dt.bfloat16
    f8 = mybir.dt.float8e4
    AF = mybir.ActivationFunctionType
    Alu = mybir.AluOpType

    nc = bacc.Bacc()

    # x params are host-laid-out [p][d][c] so each DMA row is contiguous
    xm = nc.declare_dram_parameter("xm", [128, 8 * L], bf16, isOutput=False)
    xp8 = nc.declare_dram_parameter("xp8", [128, 8 * L], f8, isOutput=False)
    wqkm = nc.declare_dram_parameter("wqkm", [128, D + C], bf16,
                                     isOutput=False)
    wvk = nc.declare_dram_parameter("wvk", [128, D], bf16, isOutput=False)
    wkvp = nc.declare_dram_parameter("wkvp", [128, D], f8, isOutput=False)
    wot = nc.declare_dram_parameter("wot", [128, D], bf16, isOutput=False)
    sel = nc.declare_dram_parameter("sel", [C, 1], f32, isOutput=False)
    out = nc.declare_dram_parameter("out", [L, D], bf16, isOutput=True)

    with tile.TileContext(nc) as tc:
        with (
            tc.tile_pool(name="consts", bufs=1) as consts,
            tc.tile_pool(name="xmp", bufs=1) as xm_pool,
            tc.tile_pool(name="xpp", bufs=1) as xp_pool,
            tc.tile_pool(name="proj", bufs=1) as proj_pool,
            tc.tile_pool(name="vko", bufs=NGRP) as vko_pool,
            tc.tile_pool(name="vkop", bufs=NGRP) as vkop_pool,
            tc.tile_pool(name="small", bufs=6) as small,
            tc.tile_pool(name="tmp", bufs=6) as tmp_pool,
            tc.tile_pool(name="state_pool", bufs=1, space="PSUM") as state_pool,
        ):
            # ---- constants + x: few fat DMA calls, ordered so the first
            # group's operands land first ----
            wqkm_sb = consts.tile([128, D + C], bf16, tag="wqkm")
            wvk_sb3 = consts.tile([128, D], bf16, tag="wvk")
            wkvp_sb3 = consts.tile([128, D], f8, tag="wkvp")
            wot_sb = consts.tile([128, D], bf16, tag="wot")
            sel_sb = consts.tile([C, 1], f32, tag="sel")
            xm3 = xm_pool.tile([128, 8, L], bf16, tag="xm3")
            xp_sb = xp_pool.tile([128, 8, L], f8, tag="xp")
            xmv = xm[:, :].rearrange("p (d c) -> p d c", d=8)
            xpv = xp8[:, :].rearrange("p (d c) -> p d c", d=8)

            nc.sync.dma_start(out=wqkm_sb, in_=wqkm[:, :])
            nc.sync.dma_start(out=xm3[:, :, 0:G], in_=xmv[:, :, 0:G])
            nc.sync.dma_start(out=wvk_sb3, in_=wvk[:, :])
            nc.sync.dma_start(out=xm3[:, :, G:2 * G], in_=xmv[:, :, G:2 * G])
            nc.sync.dma_start(out=xp_sb[:, :, 0:D], in_=xpv[:, :, 0:D])
            nc.sync.dma_start(out=wkvp_sb3, in_=wkvp[:, :])
            nc.sync.dma_start(out=xm3[:, :, D:L], in_=xmv[:, :, D:L])
            nc.sync.dma_start(out=xp_sb[:, :, D:L], in_=xpv[:, :, D:L])
            nc.sync.dma_start(out=wot_sb, in_=wot[:, :])
            nc.sync.dma_start(out=sel_sb, in_=sel[:, :])

            wqk_sb = [wqkm_sb[:, d * 128:(d + 1) * 128] for d in range(NDC)]
            wvk_sb = [wvk_sb3[:, d * 128:(d + 1) * 128] for d in range(NDC)]
            wkvp_sb = [wkvp_sb3[:, d * 128:(d + 1) * 128] for d in range(NDC)]
            mask_sb = wqkm_sb[:, D:D + C]

            # on-device constant rows for the bias matmuls
            onesr = consts.tile([1, G], bf16, tag="onesr")
            nc.vector.memset(onesr, 1.0)
            vkb = consts.tile([1, 128], bf16, tag="vkb")
            nc.vector.memset(vkb[:, 0:K], 0.0)
            nc.vector.memset(vkb[:, K:128], 1.0)
            vkbp = consts.tile([1, 128], bf16, tag="vkbp")
            nc.vector.memset(vkbp[:, 0:K], 0.0)
            nc.vector.memset(vkbp[:, K:128], XS * WS)
            ones1 = consts.tile([1, 1], bf16, tag="ones1")
            nc.vector.memset(ones1, 1.0)
            onec_sb = consts.tile([C, 1], bf16, tag="onec")
            nc.vector.memset(onec_sb, 1.0)
            bm1 = consts.tile([128, 1], f32, tag="bm1")
            nc.vector.memset(bm1, -1.0)

            # persistent sbuf; q/k/ks/attn are zero-padded to 128 partitions
            # so every Phase B matmul contracts over the full PE array
            qT_sb = proj_pool.tile([128, L], bf16, tag="qT")
            kT_sb = proj_pool.tile([128, L], bf16, tag="kT")
            attn_all = proj_pool.tile([128, NCHUNK * C], bf16, tag="attn")
            nc.vector.memset(qT_sb[K:128, :], 0.0)
            nc.vector.memset(kT_sb[K:128, :], 0.0)
            nc.vector.memset(attn_all[K:128, :], 0.0)
            vkos = [vko_pool.tile([C, 4, 130], bf16, tag=f"vko{g}",
                                  name=f"vko{g}") for g in range(NGRP)]
            vkps = [vkop_pool.tile([C, 4, 130], bf16, tag=f"vkp{g}",
                                   name=f"vkp{g}") for g in range(NGRP)]
            kfss = [vkop_pool.tile([C, 4, K], bf16, tag=f"kfs{g}",
                                   name=f"kfs{g}") for g in range(NGRP)]
            ks_sb = small.tile([128, K + 1], bf16, tag="ks")
            nc.vector.memset(ks_sb[K:128, :], 0.0)

            # running state [K, K+1]: cols 0:K = S[k,m], col K = k_sum.
            state_ps = state_pool.tile([K, 1 + K], f32)

            def tok_major(xt, wt, bias_row, vko_g, g, dtype_note):
                """[V|1|K] token-major projection for one 4-chunk group."""
                pp = None
                for c4 in range(4):
                    sl = slice((g * 4 + c4) * C, (g * 4 + c4 + 1) * C)
                    if c4 == 0:
                        pp = pp_pool.tile([C, 4, 128], f32, tag="pp",
                                          name="pp")
                    for d in range(NDC):
                        nc.tensor.matmul(pp[:, c4, :], xt[:, d, sl], wt[d],
                                         start=(c4 == 0 and d == 0),
                                         stop=False, skip_group_check=True)
                    nc.tensor.matmul(pp[:, c4, :], onesr[:, 0:C], bias_row,
                                     start=False, stop=(c4 == 3),
                                     skip_group_check=True)
                return pp

            # =============== PHASE A: projections ===============
            with (
                tc.tile_pool(name="p1_ps", bufs=2, space="PSUM") as p1_pool,
                tc.tile_pool(name="pp_ps", bufs=2, space="PSUM") as pp_pool,
                tc.tile_pool(name="ppp_ps", bufs=2, space="PSUM") as ppp_pool,
            ):
                for g in range(NGRP):
                    gs = slice(g * G, (g + 1) * G)
                    # ---- [Q|K] feature-major, bf16; +1 via ones matmul ----
                    p1 = p1_pool.tile([128, G], f32, tag="p1")
                    for d in range(NDC):
                        nc.tensor.matmul(p1, wqk_sb[d], xm3[:, d, gs],
                                         start=(d == 0), stop=False)
                    nc.tensor.matmul(p1, ones1[:, 0:1].to_broadcast((1, 128)),
                                     onesr, start=False, stop=True)
                    e1 = tmp_pool.tile([128, G], f32, tag="e1")
                    nc.scalar.activation(e1, p1, AF.Exp, bias=bm1)
                    nc.vector.scalar_tensor_tensor(
                        qT_sb[0:K, gs], e1[0:K, :], 1.0, p1[0:K, :],
                        op0=Alu.min, op1=Alu.max)
                    nc.vector.scalar_tensor_tensor(
                        kT_sb[0:K, gs], e1[K:2 * K, :], 1.0, p1[K:2 * K, :],
                        op0=Alu.min, op1=Alu.max)

                    # ---- [V|1|K] token-major main, bf16 ----
                    pp = tok_major(xm3, wvk_sb, vkb, vkos[g], g, "main")
                    vg = vkos[g]
                    nc.scalar.copy(vg[:, :, 0:K], pp[:, :, 0:K])
                    nc.vector.memset(vg[:, :, K:K + 1], 1.0)
                    e3 = tmp_pool.tile([C, 4, K], f32, tag="e3")
                    nc.scalar.activation(e3, pp[:, :, K:128], AF.Exp,
                                         bias=bm1)
                    nc.vector.scalar_tensor_tensor(
                        vg[:, :, K + 1:2 * K + 1], e3, 1.0, pp[:, :, K:128],
                        op0=Alu.min, op1=Alu.max)

                    # ---- [V|1|K] token-major prefix, fp8 (x16/W256) ----
                    ppx = None
                    for c4 in range(4):
                        sl = slice((g * 4 + c4) * C, (g * 4 + c4 + 1) * C)
                        if c4 == 0:
                            ppx = ppp_pool.tile([C, 4, 128], f32, tag="ppp",
                                                name="ppx")
                        for d in range(NDC):
                            nc.tensor.matmul(ppx[:, c4, :], xp_sb[:, d, sl],
                                             wkvp_sb[d],
                                             start=(c4 == 0 and d == 0),
                                             stop=False,
                                             skip_group_check=True)
                        nc.tensor.matmul(ppx[:, c4, :], onesr[:, 0:C], vkbp,
                                         start=False, stop=(c4 == 3),
                                         skip_group_check=True)
                    vp = vkps[g]
                    nc.scalar.mul(vp[:, :, 0:K], ppx[:, :, 0:K], PSC)
                    nc.vector.memset(vp[:, :, K:K + 1], 1.0)
                    e4 = tmp_pool.tile([C, 4, K], f32, tag="e4")
                    nc.scalar.activation(e4, ppx[:, :, K:128], AF.Exp,
                                         scale=PSC, bias=bm1)
                    e4m = tmp_pool.tile([C, 4, K], f32, tag="e4m")
                    nc.vector.tensor_scalar_min(e4m, e4, 1.0)
                    nc.vector.scalar_tensor_tensor(
                        vp[:, :, K + 1:2 * K + 1], ppx[:, :, K:128], PSC,
                        e4m, op0=Alu.mult, op1=Alu.max)
                    # kf * sel for the state accumulation
                    nc.vector.tensor_scalar_mul(
                        kfss[g], vp[:, :, K + 1:2 * K + 1], sel_sb)

                    # prefix state accumulation (lagged one group)
                    if g > 0:
                        for c4 in range(4):
                            ci = (g - 1) * 4 + c4
                            nc.tensor.matmul(
                                state_ps, kfss[g - 1][:, c4, :],
                                vkps[g - 1][:, c4, 0:K + 1],
                                start=(ci == 0), stop=False,
                                skip_group_check=True)
                for c4 in range(4):
                    nc.tensor.matmul(state_ps, kfss[NGRP - 1][:, c4, :],
                                     vkps[NGRP - 1][:, c4, 0:K + 1],
                                     start=False, stop=False,
                                     skip_group_check=True)

            # sbuf copy of the running state used as matmul lhsT
            nc.scalar.copy(ks_sb[0:K, :], state_ps)

            # =============== PHASE B: attention ===============
            with (
                tc.tile_pool(name="andc_ps", bufs=4, space="PSUM") as an_pool,
                tc.tile_pool(name="op_ps", bufs=3, space="PSUM") as op_pool,
                tc.tile_pool(name="ostage", bufs=3) as ostage_pool,
            ):
                ats = [None] * NCHUNK
                atms = [None] * NCHUNK
                nds = [None] * NCHUNK
                dcs = [None] * NCHUNK
                recs = [None] * NCHUNK

                def vko_sl(i, a, b):
                    return vkos[i // 4][:, i % 4, a:b]

                def sc(i):
                    sl = slice(i * C, (i + 1) * C)
                    ats[i] = an_pool.tile([C, C], f32, tag="andc", name="at")
                    nc.tensor.matmul(ats[i], kT_sb[:, sl], qT_sb[:, sl],
                                     start=True, stop=True)

                def vecatm(i):
                    atms[i] = tmp_pool.tile([C, C], bf16, tag="atm",
                                            name="atm")
                    nc.vector.tensor_tensor(atms[i], ats[i], mask_sb, Alu.mult)

                def nd(i):
                    # feature-major numerator [K, C] + den column [C, 1]
                    sl = slice(i * C, (i + 1) * C)
                    nds[i] = an_pool.tile([K, C], f32, tag="andc", name="nd")
                    nc.tensor.matmul(nds[i], vko_sl(i, 0, K), atms[i],
                                     start=True, stop=False)
                    nc.tensor.matmul(nds[i], ks_sb[:, 0:K], qT_sb[:, sl],
                                     start=False, stop=True)
                    dcs[i] = an_pool.tile([C, 1], f32, tag="andc", name="dc")
                    nc.tensor.matmul(dcs[i], atms[i], onec_sb,
                                     start=True, stop=False)
                    nc.tensor.matmul(dcs[i], qT_sb[:, sl], ks_sb[:, K:K + 1],
                                     start=False, stop=True)

                def st(i):
                    nc.tensor.matmul(state_ps, vko_sl(i, K + 1, 2 * K + 1),
                                     vko_sl(i, 0, K + 1),
                                     start=False, stop=(i == NCHUNK - 1),
                                     skip_group_check=True)

                def ksc(i):
                    if i < NCHUNK - 1:
                        nc.scalar.copy(ks_sb[0:K, :], state_ps)

                def recattn(i):
                    recs[i] = small.tile([C, 1], f32, tag="rec", name="rec")
                    nc.vector.reciprocal(recs[i], dcs[i])
                    nc.scalar.copy(attn_all[0:K, i * C:(i + 1) * C], nds[i])

                def op(i):
                    asl = attn_all[:, i * C:(i + 1) * C]
                    ost = ostage_pool.tile([C, D], bf16, tag="ost",
                                           name="ost")
                    o1 = op_pool.tile([C, D // 2], f32, tag="op", name="op")
                    nc.tensor.matmul(o1, asl, wot_sb[:, 0:512],
                                     start=True, stop=True)
                    o2 = op_pool.tile([C, D // 2], f32, tag="op", name="op")
                    nc.tensor.matmul(o2, asl, wot_sb[:, 512:1024],
                                     start=True, stop=True)
                    nc.scalar.activation(ost[:, 0:512], o1, AF.Copy,
                                         scale=recs[i])
                    nc.vector.tensor_scalar_mul(ost[:, 512:1024], o2,
                                                recs[i])
                    nc.sync.dma_start(out=out[i * C:(i + 1) * C, :], in_=ost)

                sc(0)
                vecatm(0)
                for i in range(NCHUNK):
                    nd(i)
                    st(i)
                    if i + 1 < NCHUNK:
                        sc(i + 1)
                    recattn(i)
                    if i + 1 < NCHUNK:
                        vecatm(i + 1)
                    ksc(i)
                    if i >= 1:
                        op(i - 1)
                op(NCHUNK - 1)

    nc.compile()
    worst = []
    for fn in nc.m.functions:
        for blk in fn.blocks:
            for inst in blk.instructions:
                n = len(inst.sync_info.on_wait) if inst.sync_info else 0
                if n > 1 and type(inst).__name__ == "InstMatmult":
                    worst.append((inst.name, n))
    if worst:
        import sys
        print(f"WARN: matmuls with >1 wait after lowering: {worst}",
              file=sys.stderr)
    return nc


def _prep_inputs(x, Wq, Wk, Wv, Wo):
    import ml_dtypes

    bf16 = ml_dtypes.bfloat16
    f8 = ml_dtypes.float8_e4m3

    def dmajor(Wcat):
        # [p, d*128+m] = Wcat[m, 128*d+p]
        return np.ascontiguousarray(
            Wcat.T.reshape(NDC, 128, 128).transpose(1, 0, 2).reshape(128, D))

    wqk = dmajor(np.concatenate([Wq, Wk], axis=0))           # [Q|K]
    wvkc = dmajor(np.concatenate([Wv, Wk], axis=0))          # [V|K]
    mask = np.triu(np.ones((C, C), np.float32))              # keep s <= t
    wqkm = np.concatenate([wqk, mask], axis=1).astype(bf16)
    wvk_h = wvkc.astype(bf16)
    wkvp = (WS * wvkc).astype(f8)                            # prefix, scaled
    wot = np.concatenate([Wo.T, np.zeros((D - K, D), np.float32)],
                         axis=0).astype(bf16)                # [128, D] padded
    zeros_xp = np.zeros((128, 8 * L), dtype=f8)

    def pdc(xt):
        # [D, L] -> [p][d][c] flat [128, 8*L]
        return np.ascontiguousarray(
            xt.reshape(NDC, 128, L).transpose(1, 0, 2).reshape(128, 8 * L))

    in_maps = []
    for core in range(8):
        b, h = core // 2, core % 2
        xb = x[b].T                                          # [D, S]
        if h:
            xp = pdc(XS * xb[:, 0:L]).astype(f8)
        else:
            xp = zeros_xp
        m = {
            "xm": pdc(xb[:, h * L:(h + 1) * L].astype(bf16)),
            "xp8": xp,
            "wqkm": wqkm,
            "wvk": wvk_h,
            "wkvp": wkvp,
            "wot": wot,
            "sel": np.full((C, 1), float(h), np.float32),
        }
        in_maps.append(m)
    return in_maps


def _run(inputs, trace=False):
    from concourse.bass_utils import run_bass_kernel_spmd

    if "nc" not in _cache:
        _cache["nc"] = _build_nc()
    nc = _cache["nc"]
    in_maps = _prep_inputs(
        np.asarray(inputs["x"], np.float32),
        np.asarray(inputs["Wq"], np.float32),
        np.asarray(inputs["Wk"], np.float32),
        np.asarray(inputs["Wv"], np.float32),
        np.asarray(inputs["Wo"], np.float32),
    )
    res = run_bass_kernel_spmd(nc, in_maps, list(range(8)), trace=trace)
    out = np.empty((B, S, D), np.float32)
    for core in range(8):
        b, h = core // 2, core % 2
        out[b, h * L:(h + 1) * L, :] = res.results[core]["out"].astype(
            np.float32)
    return out, res


def kernel(**inputs) -> np.ndarray:
    out, _ = _run(inputs, trace=False)
    return out


# revision 15
# speedup vs baseline: 1.4391x; 1.1149x over previous
"""Low-rank linear attention (causal, elu+1 feature map) on 8 trn2 cores.

Sharding: core = 2*b + h  (batch b in 0..3, sequence half h in 0..1).
Each core computes out[b, h*2048:(h+1)*2048, :].  Second-half cores
recompute the running K^T.V state over their 2048-token prefix on device
(sel scales the prefix contribution to zero on first-half cores so one
SPMD program serves all 8 cores).

v4 notes:
  - few fat DMA calls (multi-KB contiguous rows spread row-descriptors
    over all 16 queues; each dma_start costs ~0.5us of issue time).
  - all three projection passes stream rhs at full 128-row contraction;
    prefix runs token-major in fp8 e4m3 (x16 / W256 scaling) so no
    transposes are needed.
  - elu+1's "+1" lands in PSUM via ones-outer-product matmuls so
    evictions read PSUM directly.
  - every Phase B matmul is zero-padded to 128 contraction rows: the
    PE HAM clock gate watches array activity, and 64-row matmuls leave
    the clock throttled at 1.2 GHz.
  - 1/den folds into the output-projection evictions (per-partition
    scale on scalar + vector engines).

Shapes (hardcoded): B=4, S=4096, D=1024, K=64.  L = S/2 = 2048 tokens
per core, processed in 16 chunks of C=128.
"""

import numpy as np

B, S, D, K = 4, 4096, 1024, 64
L = S // 2          # tokens per core (main), also prefix length
C = 128             # chunk (tokens)
G = 512             # token group (4 chunks share one PSUM bank / evictions)
NCHUNK = L // C     # 16
NGRP = L // G       # 4
NDC = D // 128      # 8 contraction chunks
XS = 16.0           # prefix x fp8 scale
WS = 256.0          # prefix W fp8 scale
PSC = 1.0 / (XS * WS)

_cache = {}


def _build_nc():
    import concourse.bacc as bacc
    import concourse.tile as tile
    from concourse import mybir

    f32 = mybir.dt.float32
    bf16 = mybir.dt.bfloat16
    f8 = mybir.dt.float8e4
    AF = mybir.ActivationFunctionType
    Alu = mybir.AluOpType

    nc = bacc.Bacc()

    # x params are host-laid-out [p][d][c] so each DMA row is contiguous
    xm = nc.declare_dram_parameter("xm", [128, 8 * L], bf16, isOutput=False)
    xp8 = nc.declare_dram_parameter("xp8", [128, 8 * L], f8, isOutput=False)
    wqkm = nc.declare_dram_parameter("wqkm", [128, D + C], bf16,
                                     isOutput=False)
    wvk = nc.declare_dram_parameter("wvk", [128, D], bf16, isOutput=False)
    wkvp = nc.declare_dram_parameter("wkvp", [128, D], f8, isOutput=False)
    wot = nc.declare_dram_parameter("wot", [128, D], bf16, isOutput=False)
    sel = nc.declare_dram_parameter("sel", [C, 1], f32, isOutput=False)
    out = nc.declare_dram_parameter("out", [L, D], bf16, isOutput=True)

    with tile.TileContext(nc) as tc:
        with (
            tc.tile_pool(name="consts", bufs=1) as consts,
            tc.tile_pool(name="xmp", bufs=1) as xm_pool,
            tc.tile_pool(name="xpp", bufs=1) as xp_pool,
            tc.tile_pool(name="proj", bufs=1) as proj_pool,
            tc.tile_pool(name="vko", bufs=NGRP) as vko_pool,
            tc.tile_pool(name="vkop", bufs=NGRP) as vkop_pool,
            tc.tile_pool(name="small", bufs=6) as small,
            tc.tile_pool(name="tmp", bufs=6) as tmp_pool,
            tc.tile_pool(name="state_pool", bufs=1, space="PSUM") as state_pool,
        ):
            # ---- constants + x: few fat DMA calls, ordered so the first
            # group's operands land first ----
            wqkm_sb = consts.tile([128, D + C], bf16, tag="wqkm")
            wvk_sb3 = consts.tile([128, D], bf16, tag="wvk")
            wkvp_sb3 = consts.tile([128, D], f8, tag="wkvp")
            wot_sb = consts.tile([128, D], bf16, tag="wot")
            sel_sb = consts.tile([C, 1], f32, tag="sel")
            xm3 = xm_pool.tile([128, 8, L], bf16, tag="xm3")
            xp_sb = xp_pool.tile([128, 8, L], f8, tag="xp")
            xmv = xm[:, :].rearrange("p (d c) -> p d c", d=8)
            xpv = xp8[:, :].rearrange("p (d c) -> p d c", d=8)

            nc.sync.dma_start(out=wqkm_sb, in_=wqkm[:, :])
            nc.sync.dma_start(out=xm3[:, :, 0:G], in_=xmv[:, :, 0:G])
            nc.sync.dma_start(out=wvk_sb3, in_=wvk[:, :])
            nc.sync.dma_start(out=xm3[:, :, G:2 * G], in_=xmv[:, :, G:2 * G])
            nc.sync.dma_start(out=xp_sb[:, :, 0:D], in_=xpv[:, :, 0:D])
            nc.sync.dma_start(out=wkvp_sb3, in_=wkvp[:, :])
            nc.sync.dma_start(out=xm3[:, :, D:L], in_=xmv[:, :, D:L])
            nc.sync.dma_start(out=xp_sb[:, :, D:L], in_=xpv[:, :, D:L])
            nc.sync.dma_start(out=wot_sb, in_=wot[:, :])
            nc.sync.dma_start(out=sel_sb, in_=sel[:, :])

            wqk_sb = [wqkm_sb[:, d * 128:(d + 1) * 128] for d in range(NDC)]
            wvk_sb = [wvk_sb3[:, d * 128:(d + 1) * 128] for d in range(NDC)]
            wkvp_sb = [wkvp_sb3[:, d * 128:(d + 1) * 128] for d in range(NDC)]
            mask_sb = wqkm_sb[:, D:D + C]

            # on-device constant rows for the bias matmuls
            onesr = consts.tile([1, G], bf16, tag="onesr")
            nc.vector.memset(onesr, 1.0)
            vkb = consts.tile([1, 128], bf16, tag="vkb")
            nc.vector.memset(vkb[:, 0:K], 0.0)
            nc.vector.memset(vkb[:, K:128], 1.0)
            vkbp = consts.tile([1, 128], bf16, tag="vkbp")
            nc.vector.memset(vkbp[:, 0:K], 0.0)
            nc.vector.memset(vkbp[:, K:128], XS * WS)
            ones1 = consts.tile([1, 1], bf16, tag="ones1")
            nc.vector.memset(ones1, 1.0)
            onec_sb = consts.tile([C, 1], bf16, tag="onec")
            nc.vector.memset(onec_sb, 1.0)
            bm1 = consts.tile([128, 1], f32, tag="bm1")
            nc.vector.memset(bm1, -1.0)

            # persistent sbuf; q/k/ks/attn are zero-padded to 128 partitions
            # so every Phase B matmul contracts over the full PE array
            qT_sb = proj_pool.tile([128, L], bf16, tag="qT")
            kT_sb = proj_pool.tile([128, L], bf16, tag="kT")
            attn_all = proj_pool.tile([128, NCHUNK * C], bf16, tag="attn")
            nc.vector.memset(qT_sb[K:128, :], 0.0)
            nc.vector.memset(kT_sb[K:128, :], 0.0)
            nc.vector.memset(attn_all[K:128, :], 0.0)
            vkos = [vko_pool.tile([C, 4, 130], bf16, tag=f"vko{g}",
                                  name=f"vko{g}") for g in range(NGRP)]
            vkps = [vkop_pool.tile([C, 4, 130], bf16, tag=f"vkp{g}",
                                   name=f"vkp{g}") for g in range(NGRP)]
            kfss = [vkop_pool.tile([C, 4, K], bf16, tag=f"kfs{g}",
                                   name=f"kfs{g}") for g in range(NGRP)]
            ks_sb = small.tile([128, K + 1], bf16, tag="ks")
            nc.vector.memset(ks_sb[K:128, :], 0.0)

            # running state [K, K+1]: cols 0:K = S[k,m], col K = k_sum.
            state_ps = state_pool.tile([K, 1 + K], f32)

            def tok_major(xt, wt, bias_row, vko_g, g, dtype_note):
                """[V|1|K] token-major projection for one 4-chunk group."""
                pp = None
                for c4 in range(4):
                    sl = slice((g * 4 + c4) * C, (g * 4 + c4 + 1) * C)
                    if c4 == 0:
                        pp = pp_pool.tile([C, 4, 128], f32, tag="pp",
                                          name="pp")
                    for d in range(NDC):
                        nc.tensor.matmul(pp[:, c4, :], xt[:, d, sl], wt[d],
                                         start=(c4 == 0 and d == 0),
                                         stop=False, skip_group_check=True)
                    nc.tensor.matmul(pp[:, c4, :], onesr[:, 0:C], bias_row,
                                     start=False, stop=(c4 == 3),
                                     skip_group_check=True)
                return pp

            # =============== PHASE A+B interleaved ===============
            # Phase-1: group-0 main projections + the whole prefix (fp8
            # token-major) + prefix state.  Phase-2: remaining main
            # projections with attention chunks woven between them so the
            # PE stream never idles (the HAM clock gate throttles the PE
            # to 1.2 GHz after ~1us of idle and never recovers).
            with (
                tc.tile_pool(name="p1_ps", bufs=1, space="PSUM") as p1_pool,
                tc.tile_pool(name="pp_ps", bufs=1, space="PSUM") as pp_pool,
                tc.tile_pool(name="ostage", bufs=3) as ostage_pool,
            ):
                def qk_group(g):
                    gs = slice(g * G, (g + 1) * G)
                    p1 = p1_pool.tile([128, G], f32, tag="p1", name="p1")
                    for d in range(NDC):
                        nc.tensor.matmul(p1, wqk_sb[d], xm3[:, d, gs],
                                         start=(d == 0), stop=False)
                    nc.tensor.matmul(p1, ones1[:, 0:1].to_broadcast((1, 128)),
                                     onesr, start=False, stop=True)
                    e1 = tmp_pool.tile([128, G], f32, tag="e1", name="e1")
                    nc.scalar.activation(e1, p1, AF.Exp, bias=bm1)
                    nc.vector.scalar_tensor_tensor(
                        qT_sb[0:K, gs], e1[0:K, :], 1.0, p1[0:K, :],
                        op0=Alu.min, op1=Alu.max)
                    nc.vector.scalar_tensor_tensor(
                        kT_sb[0:K, gs], e1[K:2 * K, :], 1.0, p1[K:2 * K, :],
                        op0=Alu.min, op1=Alu.max)

                def kv_group(g):
                    pp = pp_pool.tile([C, 4, 128], f32, tag="pp", name="pp")
                    for c4 in range(4):
                        sl = slice((g * 4 + c4) * C, (g * 4 + c4 + 1) * C)
                        for d in range(NDC):
                            nc.tensor.matmul(pp[:, c4, :], xm3[:, d, sl],
                                             wvk_sb[d],
                                             start=(c4 == 0 and d == 0),
                                             stop=False,
                                             skip_group_check=True)
                        nc.tensor.matmul(pp[:, c4, :], onesr[:, 0:C], vkb,
                                         start=False, stop=(c4 == 3),
                                         skip_group_check=True)
                    vg = vkos[g]
                    nc.scalar.copy(vg[:, :, 0:K], pp[:, :, 0:K])
                    nc.vector.memset(vg[:, :, K:K + 1], 1.0)
                    e3 = tmp_pool.tile([C, 4, K], f32, tag="e3", name="e3")
                    nc.scalar.activation(e3, pp[:, :, K:128], AF.Exp,
                                         bias=bm1)
                    nc.vector.scalar_tensor_tensor(
                        vg[:, :, K + 1:2 * K + 1], e3, 1.0, pp[:, :, K:128],
                        op0=Alu.min, op1=Alu.max)

                # ---- Phase B helpers ----
                ats = [None] * NCHUNK
                atms = [None] * NCHUNK
                nds = [None] * NCHUNK
                dcs = [None] * NCHUNK
                recs = [None] * NCHUNK

                def vko_sl(i, a, b):
                    return vkos[i // 4][:, i % 4, a:b]

                an_pool = []
                op_pool = []

                def sc(i):
                    sl = slice(i * C, (i + 1) * C)
                    ats[i] = an_pool[0].tile([C, C], f32, tag="andc",
                                             name="at")
                    nc.tensor.matmul(ats[i], kT_sb[:, sl], qT_sb[:, sl],
                                     start=True, stop=True)

                def vecatm(i):
                    atms[i] = tmp_pool.tile([C, C], bf16, tag="atm",
                                            name="atm")
                    nc.vector.tensor_tensor(atms[i], ats[i], mask_sb, Alu.mult)

                def nd(i):
                    sl = slice(i * C, (i + 1) * C)
                    nds[i] = an_pool[0].tile([K, C], f32, tag="andc",
                                             name="nd")
                    nc.tensor.matmul(nds[i], vko_sl(i, 0, K), atms[i],
                                     start=True, stop=False)
                    nc.tensor.matmul(nds[i], ks_sb[:, 0:K], qT_sb[:, sl],
                                     start=False, stop=True)
                    dcs[i] = an_pool[0].tile([C, 1], f32, tag="andc",
                                             name="dc")
                    nc.tensor.matmul(dcs[i], atms[i], onec_sb,
                                     start=True, stop=False)
                    nc.tensor.matmul(dcs[i], qT_sb[:, sl], ks_sb[:, K:K + 1],
                                     start=False, stop=True)

                def st(i):
                    nc.tensor.matmul(state_ps, vko_sl(i, K + 1, 2 * K + 1),
                                     vko_sl(i, 0, K + 1),
                                     start=False, stop=(i == NCHUNK - 1),
                                     skip_group_check=True)

                def ksc(i):
                    if i < NCHUNK - 1:
                        nc.scalar.copy(ks_sb[0:K, :], state_ps)

                def recattn(i):
                    recs[i] = small.tile([C, 1], f32, tag="rec", name="rec")
                    nc.vector.reciprocal(recs[i], dcs[i])
                    nc.scalar.copy(attn_all[0:K, i * C:(i + 1) * C], nds[i])

                def op(i):
                    asl = attn_all[:, i * C:(i + 1) * C]
                    ost = ostage_pool.tile([C, D], bf16, tag="ost",
                                           name="ost")
                    o1 = op_pool[0].tile([C, D // 2], f32, tag="op",
                                         name="op")
                    nc.tensor.matmul(o1, asl, wot_sb[:, 0:512],
                                     start=True, stop=True)
                    o2 = op_pool[0].tile([C, D // 2], f32, tag="op",
                                         name="op")
                    nc.tensor.matmul(o2, asl, wot_sb[:, 512:1024],
                                     start=True, stop=True)
                    nc.scalar.activation(ost[:, 0:512], o1, AF.Copy,
                                         scale=recs[i])
                    nc.vector.tensor_scalar_mul(ost[:, 512:1024], o2,
                                                recs[i])
                    nc.sync.dma_start(out=out[i * C:(i + 1) * C, :], in_=ost)

                def emit_b(i):
                    nd(i)
                    st(i)
                    if i + 1 < NCHUNK:
                        sc(i + 1)
                    recattn(i)
                    if i + 1 < NCHUNK:
                        vecatm(i + 1)
                    ksc(i)
                    if i >= 1:
                        op(i - 1)

                # ---- phase-1 ----
                with tc.tile_pool(name="ppp_ps", bufs=2,
                                  space="PSUM") as ppp_pool:
                    qk_group(0)
                    kv_group(0)
                    for g in range(NGRP):
                        ppx = ppp_pool.tile([C, 4, 128], f32, tag="ppp",
                                            name="ppx")
                        for c4 in range(4):
                            sl = slice((g * 4 + c4) * C,
                                       (g * 4 + c4 + 1) * C)
                            for d in range(NDC):
                                nc.tensor.matmul(ppx[:, c4, :],
                                                 xp_sb[:, d, sl],
                                                 wkvp_sb[d],
                                                 start=(c4 == 0 and d == 0),
                                                 stop=False,
                                                 skip_group_check=True)
                            nc.tensor.matmul(ppx[:, c4, :], onesr[:, 0:C],
                                             vkbp, start=False,
                                             stop=(c4 == 3),
                                             skip_group_check=True)
                        vp = vkps[g]
                        nc.scalar.mul(vp[:, :, 0:K], ppx[:, :, 0:K], PSC)
                        nc.vector.memset(vp[:, :, K:K + 1], 1.0)
                        e4 = tmp_pool.tile([C, 4, K], f32, tag="e4",
                                           name="e4")
                        nc.scalar.activation(e4, ppx[:, :, K:128], AF.Exp,
                                             scale=PSC, bias=bm1)
                        e4m = tmp_pool.tile([C, 4, K], f32, tag="e4m",
                                            name="e4m")
                        nc.vector.tensor_scalar_min(e4m, e4, 1.0)
                        nc.vector.scalar_tensor_tensor(
                            vp[:, :, K + 1:2 * K + 1], ppx[:, :, K:128], PSC,
                            e4m, op0=Alu.mult, op1=Alu.max)
                        nc.vector.tensor_scalar_mul(
                            kfss[g], vp[:, :, K + 1:2 * K + 1], sel_sb)
                        if g > 0:
                            for c4 in range(4):
                                ci = (g - 1) * 4 + c4
                                nc.tensor.matmul(
                                    state_ps, kfss[g - 1][:, c4, :],
                                    vkps[g - 1][:, c4, 0:K + 1],
                                    start=(ci == 0), stop=False,
                                    skip_group_check=True)
                    for c4 in range(4):
                        nc.tensor.matmul(state_ps, kfss[NGRP - 1][:, c4, :],
                                         vkps[NGRP - 1][:, c4, 0:K + 1],
                                         start=False, stop=False,
                                         skip_group_check=True)
                    nc.scalar.copy(ks_sb[0:K, :], state_ps)

                # ---- phase-2: weave attention chunks between projection
                # groups so the PE never idles ----
                with (
                    tc.tile_pool(name="andc_ps", bufs=3,
                                 space="PSUM") as an_pool2,
                    tc.tile_pool(name="op_ps", bufs=2,
                                 space="PSUM") as op_pool2,
                ):
                    an_pool.append(an_pool2)
                    op_pool.append(op_pool2)
                    sc(0)
                    vecatm(0)
                    for g in range(1, NGRP):
                        qk_group(g)
                        emit_b(4 * (g - 1))
                        emit_b(4 * (g - 1) + 1)
                        kv_group(g)
                        emit_b(4 * (g - 1) + 2)
                        emit_b(4 * (g - 1) + 3)
                    for i in range(4 * (NGRP - 1), NCHUNK):
                        emit_b(i)
                    op(NCHUNK - 1)

    nc.compile()
    worst = []
    for fn in nc.m.functions:
        for blk in fn.blocks:
            for inst in blk.instructions:
                n = len(inst.sync_info.on_wait) if inst.sync_info else 0
                if n > 1 and type(inst).__name__ == "InstMatmult":
                    worst.append((inst.name, n))
    if worst:
        import sys
        print(f"WARN: matmuls with >1 wait after lowering: {worst}",
              file=sys.stderr)
    return nc


def _prep_inputs(x, Wq, Wk, Wv, Wo):
    import ml_dtypes

    bf16 = ml_dtypes.bfloat16
    f8 = ml_dtypes.float8_e4m3

    def dmajor(Wcat):
        # [p, d*128+m] = Wcat[m, 128*d+p]
        return np.ascontiguousarray(
            Wcat.T.reshape(NDC, 128, 128).transpose(1, 0, 2).reshape(128, D))

    wqk = dmajor(np.concatenate([Wq, Wk], axis=0))           # [Q|K]
    wvkc = dmajor(np.concatenate([Wv, Wk], axis=0))          # [V|K]
    mask = np.triu(np.ones((C, C), np.float32))              # keep s <= t
    wqkm = np.concatenate([wqk, mask], axis=1).astype(bf16)
    wvk_h = wvkc.astype(bf16)
    wkvp = (WS * wvkc).astype(f8)                            # prefix, scaled
    wot = np.concatenate([Wo.T, np.zeros((D - K, D), np.float32)],
                         axis=0).astype(bf16)                # [128, D] padded
    zeros_xp = np.zeros((128, 8 * L), dtype=f8)

    def pdc(xt):
        # [D, L] -> [p][d][c] flat [128, 8*L]
        return np.ascontiguousarray(
            xt.reshape(NDC, 128, L).transpose(1, 0, 2).reshape(128, 8 * L))

    in_maps = []
    for core in range(8):
        b, h = core // 2, core % 2
        xb = x[b].T                                          # [D, S]
        if h:
            xp = pdc(XS * xb[:, 0:L]).astype(f8)
        else:
            xp = zeros_xp
        m = {
            "xm": pdc(xb[:, h * L:(h + 1) * L].astype(bf16)),
            "xp8": xp,
            "wqkm": wqkm,
            "wvk": wvk_h,
            "wkvp": wkvp,
            "wot": wot,
            "sel": np.full((C, 1), float(h), np.float32),
        }
        in_maps.append(m)
    return in_maps


def _run(inputs, trace=False):
    from concourse.bass_utils import run_bass_kernel_spmd

    if "nc" not in _cache:
        _cache["nc"] = _build_nc()
    nc = _cache["nc"]
    in_maps = _prep_inputs(
        np.asarray(inputs["x"], np.float32),
        np.asarray(inputs["Wq"], np.float32),
        np.asarray(inputs["Wk"], np.float32),
        np.asarray(inputs["Wv"], np.float32),
        np.asarray(inputs["Wo"], np.float32),
    )
    res = run_bass_kernel_spmd(nc, in_maps, list(range(8)), trace=trace)
    out = np.empty((B, S, D), np.float32)
    for core in range(8):
        b, h = core // 2, core % 2
        out[b, h * L:(h + 1) * L, :] = res.results[core]["out"].astype(
            np.float32)
    return out, res


def kernel(**inputs) -> np.ndarray:
    out, _ = _run(inputs, trace=False)
    return out
